# revision 10
# baseline (speedup 1.0000x reference)
"""Trainium2 Bass kernel for nn_CausalMask (gumbel-sigmoid node/edge masks +
symmetric scatter into a [P, P] edge mask), SPMD across 8 NeuronCores.

Strategy (row-sharded scatter):
  - Core k owns rows [k*768, (k+1)*768) of the [6144, 6144] edge mask.
    Its block lives in SBUF as [128 partitions x 36864 f32] (partition p
    holds mask rows 6p..6p+5 of the block, row-major).
  - The host routes each scattered entry (both (r,c) and (c,r) of every
    edge) to (core, partition, window, slot). A window is a 768-f32
    column segment of one row: 48 windows per partition.
  - On device: ACT/DVE compute the gumbel-sigmoid edge values for the
    padded entry buffer; 48 gpsimd local_scatter calls compose the block
    (zeros + values fused). f32 exactness via the int16-pair bitcast
    trick (value f32 bits land as two adjacent int16 scatters).
  - 8 large HWDGE DMAs stream the block to HBM, pipelined with the
    scatters (chunk j = windows 6j..6j+5 = one SBUF tile).
"""

import sys
import types

for _p in ("/opt/trn_rl_repo", "/root/.axon_site"):
    if _p not in sys.path:
        sys.path.insert(0, _p)

# NTFF profile hook (used only when BASS_TRACE=1): the image's antenv lacks
# axon_hooks, so provide it via sys.modules before bass_utils imports it.
if "antenv.axon_hooks" not in sys.modules:
    _m = types.ModuleType("antenv.axon_hooks")

    def _get_hook():
        try:
            from trn_agent_boot.trn_boot import _ntff_profile_via_ctypes

            return _ntff_profile_via_ctypes("/opt/axon/libaxon_pjrt.so")
        except Exception:
            return None

    _m.get_axon_ntff_profile_hook = _get_hook
    _m.set_axon_ntff_profile_hook = lambda h: None
    sys.modules["antenv.axon_hooks"] = _m

import numpy as np

P = 6144          # num_patches
E = 262144        # number of edges
NCORES = 8
RPB = P // NCORES     # 768 rows per core block
RPP = RPB // 128      # 6 mask rows per partition
NCT = 8               # column tiles per row
WF = P // NCT         # 768 f32 per window
W = RPP * NCT         # 48 windows per partition
CHW = RPP             # windows per DMA chunk (6)
NCH = W // CHW        # 8 output DMA chunks
CHF = CHW * WF        # 4608 f32 per chunk per partition
TAU = 1.0
EPS = 1e-10

_BUILD_CACHE: dict[int, object] = {}
LAST_RESULTS = None   # BassKernelResults of the most recent run (for test.py)


def _build_program(K: int):
    """Build + finalize the SPMD Bass program for per-cell slot count K."""
    import concourse.bacc as bacc
    import concourse.mybir as mybir
    import concourse.tile as tile

    f32 = mybir.dt.float32
    i16 = mybir.dt.int16
    AF = mybir.ActivationFunctionType
    ALU = mybir.AluOpType
    LK = W * K
    NPF = P // 128  # 48 node values per partition

    nc = bacc.Bacc()
    # register EPS as a const AP so activation(bias=EPS) resolves
    _ct = nc.alloc_sbuf_tensor(f"const-f32-eps", [128, 1], f32)
    nc.gpsimd.memset(_ct.ap(), EPS)
    nc.const_aps.aps[(f32, EPS)] = _ct.ap()
    nc.all_engine_barrier()

    CK = CHW * K          # padded entries per partition per chunk
    SEG = 2 * CK          # int16 elems per segment (el / eu / ei)
    CPK = 3 * SEG         # packed int16 elems per chunk per partition

    pk = nc.declare_dram_parameter("pk", [128, NCH * CPK], i16, isOutput=False)
    nl = nc.declare_dram_parameter("nl", [P], f32, isOutput=False)
    nu = nc.declare_dram_parameter("nu", [P], f32, isOutput=False)
    eb = nc.declare_dram_parameter("edge_block", [RPB, P], f32, isOutput=True)
    nm = nc.declare_dram_parameter("node_mask", [P], f32, isOutput=True)

    with tile.TileContext(nc) as tc:
        with tc.tile_pool(name="sbuf", bufs=1) as pool:
            tpk = pool.tile([128, NCH * CPK], i16, tag="tpk")

            def load_chunk(j):
                s = slice(j * CPK, (j + 1) * CPK)
                nc.scalar.dma_start(tpk[:, s], pk[:, s])

            load_chunk(0)
            load_chunk(1)

            # warm the ACT Ln/Sigmoid tables while chunk 0 is in flight
            warm = pool.tile([128, 1], f32, tag="warm")
            nc.scalar.activation(warm[:], _ct.ap(), AF.Ln, bias=EPS)
            nc.scalar.activation(warm[:], warm[:], AF.Sigmoid)

            # ---- per-chunk: compute gumbel-sigmoid values, compose windows
            #      via local_scatter, stream chunk to HBM
            ebf = eb[:, :].rearrange("(p a) b -> p (a b)", p=128)  # [128, 36864]
            for j in range(NCH):
                if j + 2 < NCH:
                    load_chunk(j + 2)
                base = j * CPK
                elf = tpk[:, base : base + SEG].bitcast(f32)
                euf = tpk[:, base + SEG : base + 2 * SEG].bitcast(f32)
                # g = -ln(-ln(u + eps) + eps);  v = sigmoid((logit + g) / tau)
                nc.scalar.activation(euf, euf, AF.Ln, bias=EPS)
                # guard: ln(u+eps) must stay <= 0 so -ln(..)+eps > 0
                nc.vector.tensor_scalar_min(euf, euf, 0.0)
                nc.scalar.activation(euf, euf, AF.Ln, bias=EPS, scale=-1.0)
                nc.vector.tensor_tensor(elf, elf, euf, op=ALU.subtract)
                nc.scalar.activation(elf, elf, AF.Sigmoid, scale=1.0 / TAU)

                blk = pool.tile([128, CHF], f32, tag=f"blk{j}")
                for wi in range(CHW):
                    nc.gpsimd.local_scatter(
                        out_ap=blk[:, wi * WF : (wi + 1) * WF].bitcast(i16),
                        data_ap=tpk[:, base + wi * 2 * K : base + (wi + 1) * 2 * K],
                        idxs_ap=tpk[
                            :,
                            base + 2 * SEG + wi * 2 * K : base + 2 * SEG + (wi + 1) * 2 * K,
                        ],
                        channels=128,
                        num_elems=2 * WF,
                        num_idxs=2 * K,
                    )
                    # stream out per 2 windows: shorter pipeline tail and
                    # earlier DMA start than one DMA per 6-window chunk
                    if wi % 2 == 1:
                        lo = (j * CHW + wi - 1) * WF
                        hi = (j * CHW + wi + 1) * WF
                        nc.sync.dma_start(
                            ebf[:, lo:hi], blk[:, (wi - 1) * WF : (wi + 1) * WF]
                        )

            # ---- node mask (identical on every core; tiny — run at the end
            #      so it never delays the scatter pipeline)
            tnl = pool.tile([128, NPF], f32, tag="tnl")
            tnu = pool.tile([128, NPF], f32, tag="tnu")
            nc.scalar.dma_start(tnl[:], nl[:].rearrange("(a b) -> a b", a=128))
            nc.scalar.dma_start(tnu[:], nu[:].rearrange("(a b) -> a b", a=128))
            nc.scalar.activation(tnu[:], tnu[:], AF.Ln, bias=EPS)
            nc.vector.tensor_scalar_min(tnu[:], tnu[:], 0.0)
            nc.scalar.activation(tnu[:], tnu[:], AF.Ln, bias=EPS, scale=-1.0)
            nc.vector.tensor_tensor(tnl[:], tnl[:], tnu[:], op=ALU.subtract)
            nc.scalar.activation(tnl[:], tnl[:], AF.Sigmoid, scale=1.0 / TAU)
            nc.sync.dma_start(nm[:].rearrange("(a b) -> a b", a=128), tnl[:])

    nc.finalize()
    return nc


def _route_entries(rows: np.ndarray, cols: np.ndarray):
    """Route 2E scattered entries to (core, partition, window, slot).

    Returns (K, dest, order) where order indexes into the concatenated
    entry list (first E: (r,c), second E: (c,r)), dest is the flat slot
    index into the per-core padded buffers [NCORES, 128, W, K], and K the
    global max entries per (core, partition, window) cell.
    """
    rr = np.concatenate([rows, cols]).astype(np.int64)
    cc = np.concatenate([cols, rows]).astype(np.int64)

    core = rr // RPB
    lr = rr - core * RPB
    p = lr // RPP
    q = lr - p * RPP
    ct = cc // WF
    cpos = cc - ct * WF
    w = q * NCT + ct
    cell = (core * 128 + p) * W + w

    order = np.argsort(cell, kind="stable")
    cell_s = cell[order]
    # rank within equal-cell runs
    first = np.r_[0, np.flatnonzero(np.diff(cell_s)) + 1]
    counts = np.diff(np.r_[first, len(cell_s)])
    K = int(counts.max())
    slot = np.arange(len(cell_s), dtype=np.int64) - np.repeat(first, counts)
    dest = cell_s * K + slot
    return K, dest, order, cpos


def kernel(node_logits, edge_logits, u_node, u_edge, rows, cols):
    global LAST_RESULTS
    from concourse.bass_utils import run_bass_kernel_spmd

    node_logits = np.asarray(node_logits, np.float32)
    edge_logits = np.asarray(edge_logits, np.float32)
    u_node = np.asarray(u_node, np.float32)
    u_edge = np.asarray(u_edge, np.float32)
    rows = np.asarray(rows)
    cols = np.asarray(cols)

    K, dest, order, cpos = _route_entries(rows, cols)

    nc = _BUILD_CACHE.get(K)
    if nc is None:
        nc = _build_program(K)
        _BUILD_CACHE[K] = nc

    # padded per-core buffers (padding values never scattered: idx = -1;
    # u=0 padding is safe through the clamped log-log pipeline)
    ncell = NCORES * 128 * W
    el_pad = np.zeros(ncell * K, np.float32)
    eu_pad = np.zeros(ncell * K, np.float32)
    ei_pad = np.full(ncell * 2 * K, -1, np.int16)

    ee = np.concatenate([np.arange(E), np.arange(E)])[order]
    el_pad[dest] = edge_logits[ee]
    eu_pad[dest] = u_edge[ee]
    cpos_s = cpos[order]
    ei_pad[2 * dest] = (2 * cpos_s).astype(np.int16)
    ei_pad[2 * dest + 1] = (2 * cpos_s + 1).astype(np.int16)

    # pack [el | eu | ei] per chunk: [NC, 128, NCH, 3, SEG] int16
    SEG = 2 * CHW * K
    el16 = el_pad.view(np.int16).reshape(NCORES, 128, NCH, SEG)
    eu16 = eu_pad.view(np.int16).reshape(NCORES, 128, NCH, SEG)
    ei16 = ei_pad.reshape(NCORES, 128, NCH, SEG)
    pk = np.stack([el16, eu16, ei16], axis=3).reshape(NCORES, 128, NCH * 3 * SEG)

    in_maps = [
        {
            "pk": pk[c],
            "nl": node_logits,
            "nu": u_node,
        }
        for c in range(NCORES)
    ]

    res = run_bass_kernel_spmd(nc, in_maps, list(range(NCORES)))
    LAST_RESULTS = res

    edge_mask = np.concatenate(
        [res.results[c]["edge_block"] for c in range(NCORES)], axis=0
    )
    node_mask = res.results[0]["node_mask"]
    return node_mask, edge_mask


# revision 11
# speedup vs baseline: 1.3403x; 1.3403x over previous
"""Trainium2 Bass kernel for nn_CausalMask (gumbel-sigmoid node/edge masks +
symmetric scatter into a [P, P] edge mask), SPMD across 8 NeuronCores.

Strategy (row-sharded scatter):
  - Core k owns rows [k*768, (k+1)*768) of the [6144, 6144] edge mask.
    Its block lives in SBUF as [128 partitions x 36864 values] (partition
    p holds mask rows 6p..6p+5 of the block, row-major).
  - The host routes each scattered entry (both (r,c) and (c,r) of every
    edge) to (core, partition, window, slot). A window is a column
    segment of one row.
  - On device: ACT/DVE compute the gumbel-sigmoid edge values for the
    padded entry buffer; gpsimd local_scatter calls compose the block
    (zeros + values fused, one window per call); DMAs stream the block
    to HBM, pipelined with the scatters.

Two composition modes:
  - exact (default): block in f32; each value lands as two adjacent int16
    scatters (bitcast trick), bit-exact f32. 48 windows of 768 cols.
  - fp16 (CM_FP16=1): block in fp16, upcast to f32 during the SWDGE
    output DMA. Half the gpsimd stream (24 windows of 1536 cols);
    max relative error 2^-11 (~4.9e-4) on scattered values.
"""

import os
import sys
import types

for _p in ("/opt/trn_rl_repo", "/root/.axon_site"):
    if _p not in sys.path:
        sys.path.insert(0, _p)

# NTFF profile hook (used only when BASS_TRACE=1): the image's antenv lacks
# axon_hooks, so provide it via sys.modules before bass_utils imports it.
if "antenv.axon_hooks" not in sys.modules:
    _m = types.ModuleType("antenv.axon_hooks")

    def _get_hook():
        try:
            from trn_agent_boot.trn_boot import _ntff_profile_via_ctypes

            return _ntff_profile_via_ctypes("/opt/axon/libaxon_pjrt.so")
        except Exception:
            return None

    _m.get_axon_ntff_profile_hook = _get_hook
    _m.set_axon_ntff_profile_hook = lambda h: None
    sys.modules["antenv.axon_hooks"] = _m

import numpy as np

P = 6144          # num_patches
E = 262144        # number of edges
NCORES = 8
RPB = P // NCORES     # 768 rows per core block
RPP = RPB // 128      # 6 mask rows per partition
BLKF = RPP * P        # 36864 values per partition
NCH = 8               # output DMA chunks (each = 4608 values/partition)
CHF = BLKF // NCH
TAU = 1.0
EPS = 1e-10

FP16 = os.environ.get("CM_FP16", "0") == "1"
if FP16:
    WF = 1536         # columns per scatter window
else:
    WF = 768
NCT = P // WF         # col tiles per row
W = RPP * NCT         # windows per partition
CHW = W // NCH        # windows per DMA chunk

_BUILD_CACHE: dict[tuple, object] = {}
LAST_RESULTS = None   # BassKernelResults of the most recent run (for test.py)


def _build_program(K: int):
    """Build + finalize the SPMD Bass program for per-cell slot count K."""
    import concourse.bacc as bacc
    import concourse.mybir as mybir
    import concourse.tile as tile

    f32 = mybir.dt.float32
    f16 = mybir.dt.float16
    i16 = mybir.dt.int16
    AF = mybir.ActivationFunctionType
    ALU = mybir.AluOpType
    LK = W * K
    NPF = P // 128  # 48 node values per partition
    IPW = K if FP16 else 2 * K    # idx int16s per window

    nc = bacc.Bacc()
    # register EPS as a const AP so activation(bias=EPS) resolves
    _ct = nc.alloc_sbuf_tensor(f"const-f32-eps", [128, 1], f32)
    nc.gpsimd.memset(_ct.ap(), EPS)
    nc.const_aps.aps[(f32, EPS)] = _ct.ap()
    nc.all_engine_barrier()

    CK = CHW * K          # padded entries per partition per chunk
    VSEG = 2 * CK         # int16 elems of el / eu segment per chunk
    ISEG = CHW * IPW      # int16 elems of idx segment per chunk
    CPK = 2 * VSEG + ISEG

    pk = nc.declare_dram_parameter("pk", [128, NCH * CPK], i16, isOutput=False)
    nl = nc.declare_dram_parameter("nl", [P], f32, isOutput=False)
    nu = nc.declare_dram_parameter("nu", [P], f32, isOutput=False)
    eb = nc.declare_dram_parameter("edge_block", [RPB, P], f32, isOutput=True)
    nm = nc.declare_dram_parameter("node_mask", [P], f32, isOutput=True)

    with tile.TileContext(nc) as tc:
        with tc.tile_pool(name="sbuf", bufs=1) as pool:
            tpk = pool.tile([128, NCH * CPK], i16, tag="tpk")
            if FP16:
                th = pool.tile([128, LK], f16, tag="th")  # downcast values

            def load_chunk(j):
                s = slice(j * CPK, (j + 1) * CPK)
                nc.scalar.dma_start(tpk[:, s], pk[:, s])

            load_chunk(0)
            load_chunk(1)

            # warm the ACT Ln/Sigmoid tables while chunk 0 is in flight
            warm = pool.tile([128, 1], f32, tag="warm")
            nc.scalar.activation(warm[:], _ct.ap(), AF.Ln, bias=EPS)
            nc.scalar.activation(warm[:], warm[:], AF.Sigmoid)

            # ---- per-chunk: compute gumbel-sigmoid values, compose windows
            #      via local_scatter, stream chunk to HBM
            ebf = eb[:, :].rearrange("(p a) b -> p (a b)", p=128)  # [128, BLKF]
            for j in range(NCH):
                if j + 2 < NCH:
                    load_chunk(j + 2)
                base = j * CPK
                elf = tpk[:, base : base + VSEG].bitcast(f32)
                euf = tpk[:, base + VSEG : base + 2 * VSEG].bitcast(f32)
                # g = -ln(-ln(u + eps) + eps);  v = sigmoid((logit + g) / tau)
                nc.scalar.activation(euf, euf, AF.Ln, bias=EPS)
                # guard: ln(u+eps) must stay <= 0 so -ln(..)+eps > 0
                nc.vector.tensor_scalar_min(euf, euf, 0.0)
                nc.scalar.activation(euf, euf, AF.Ln, bias=EPS, scale=-1.0)
                nc.vector.tensor_tensor(elf, elf, euf, op=ALU.subtract)
                nc.scalar.activation(elf, elf, AF.Sigmoid, scale=1.0 / TAU)
                if FP16:
                    hj = th[:, j * CK : (j + 1) * CK]
                    nc.vector.tensor_copy(hj, elf)

                blk = pool.tile([128, CHF], f16 if FP16 else f32, tag=f"blk{j}")
                for wi in range(CHW):
                    if FP16:
                        data = th[:, j * CK + wi * K : j * CK + (wi + 1) * K]
                        ne = WF
                    else:
                        data = tpk[:, base + wi * 2 * K : base + (wi + 1) * 2 * K]
                        ne = 2 * WF
                    nc.gpsimd.local_scatter(
                        out_ap=blk[:, wi * WF : (wi + 1) * WF].bitcast(i16),
                        data_ap=data,
                        idxs_ap=tpk[
                            :,
                            base + 2 * VSEG + wi * IPW : base + 2 * VSEG + (wi + 1) * IPW,
                        ],
                        channels=128,
                        num_elems=ne,
                        num_idxs=IPW,
                    )
                if FP16:
                    # SWDGE cast-DMA: fp16 SBUF -> f32 HBM
                    nc.gpsimd.dma_start(
                        ebf[:, j * CHF : (j + 1) * CHF], blk[:]
                    )
                else:
                    half = CHF // 2
                    nc.sync.dma_start(
                        ebf[:, j * CHF : j * CHF + half], blk[:, :half]
                    )
                    nc.sync.dma_start(
                        ebf[:, j * CHF + half : (j + 1) * CHF], blk[:, half:]
                    )

            # ---- node mask (identical on every core; tiny — run at the end
            #      so it never delays the scatter pipeline)
            tnl = pool.tile([128, NPF], f32, tag="tnl")
            tnu = pool.tile([128, NPF], f32, tag="tnu")
            nc.scalar.dma_start(tnl[:], nl[:].rearrange("(a b) -> a b", a=128))
            nc.scalar.dma_start(tnu[:], nu[:].rearrange("(a b) -> a b", a=128))
            nc.scalar.activation(tnu[:], tnu[:], AF.Ln, bias=EPS)
            nc.vector.tensor_scalar_min(tnu[:], tnu[:], 0.0)
            nc.scalar.activation(tnu[:], tnu[:], AF.Ln, bias=EPS, scale=-1.0)
            nc.vector.tensor_tensor(tnl[:], tnl[:], tnu[:], op=ALU.subtract)
            nc.scalar.activation(tnl[:], tnl[:], AF.Sigmoid, scale=1.0 / TAU)
            nc.sync.dma_start(nm[:].rearrange("(a b) -> a b", a=128), tnl[:])

    nc.finalize()
    return nc


def _route_entries(rows: np.ndarray, cols: np.ndarray):
    """Route 2E scattered entries to (core, partition, window, slot).

    Returns (K, dest, order, cpos): order indexes into the concatenated
    entry list (first E: (r,c), second E: (c,r)); dest is the flat slot
    index into the per-core padded buffers [NCORES, 128, W, K]; K is the
    global max entries per (core, partition, window) cell (even); cpos
    the in-window column position of every entry.
    """
    rr = np.concatenate([rows, cols]).astype(np.int64)
    cc = np.concatenate([cols, rows]).astype(np.int64)

    core = rr // RPB
    lr = rr - core * RPB
    p = lr // RPP
    q = lr - p * RPP
    ct = cc // WF
    cpos = cc - ct * WF
    w = q * NCT + ct
    cell = (core * 128 + p) * W + w

    order = np.argsort(cell, kind="stable")
    cell_s = cell[order]
    first = np.r_[0, np.flatnonzero(np.diff(cell_s)) + 1]
    counts = np.diff(np.r_[first, len(cell_s)])
    K = int(counts.max())
    K += K & 1  # num_idxs must be even in fp16 mode
    slot = np.arange(len(cell_s), dtype=np.int64) - np.repeat(first, counts)
    dest = cell_s * K + slot
    return K, dest, order, cpos


def kernel(node_logits, edge_logits, u_node, u_edge, rows, cols):
    global LAST_RESULTS
    from concourse.bass_utils import run_bass_kernel_spmd

    node_logits = np.asarray(node_logits, np.float32)
    edge_logits = np.asarray(edge_logits, np.float32)
    u_node = np.asarray(u_node, np.float32)
    u_edge = np.asarray(u_edge, np.float32)
    rows = np.asarray(rows)
    cols = np.asarray(cols)

    K, dest, order, cpos = _route_entries(rows, cols)

    nc = _BUILD_CACHE.get((K, FP16))
    if nc is None:
        nc = _build_program(K)
        _BUILD_CACHE[(K, FP16)] = nc

    # padded per-core buffers (padding never scattered: idx = -1; u=0
    # padding is safe through the clamped log-log pipeline)
    ncell = NCORES * 128 * W
    el_pad = np.zeros(ncell * K, np.float32)
    eu_pad = np.zeros(ncell * K, np.float32)
    IPW = K if FP16 else 2 * K
    ei_pad = np.full(ncell * IPW, -1, np.int16)

    ee = np.concatenate([np.arange(E), np.arange(E)])[order]
    el_pad[dest] = edge_logits[ee]
    eu_pad[dest] = u_edge[ee]
    cpos_s = cpos[order]
    if FP16:
        ei_pad[dest] = cpos_s.astype(np.int16)
    else:
        ei_pad[2 * dest] = (2 * cpos_s).astype(np.int16)
        ei_pad[2 * dest + 1] = (2 * cpos_s + 1).astype(np.int16)

    # pack [el | eu | ei] per chunk (int16 view, chunk-contiguous)
    VSEG = 2 * CHW * K
    ISEG = CHW * IPW
    el16 = el_pad.view(np.int16).reshape(NCORES, 128, NCH, VSEG)
    eu16 = eu_pad.view(np.int16).reshape(NCORES, 128, NCH, VSEG)
    ei16 = ei_pad.reshape(NCORES, 128, NCH, ISEG)
    CPK = 2 * VSEG + ISEG
    pk = np.empty((NCORES, 128, NCH, CPK), np.int16)
    pk[..., :VSEG] = el16
    pk[..., VSEG : 2 * VSEG] = eu16
    pk[..., 2 * VSEG :] = ei16
    pk = pk.reshape(NCORES, 128, NCH * CPK)

    in_maps = [
        {"pk": pk[c], "nl": node_logits, "nu": u_node} for c in range(NCORES)
    ]

    res = run_bass_kernel_spmd(nc, in_maps, list(range(NCORES)))
    LAST_RESULTS = res

    edge_mask = np.concatenate(
        [res.results[c]["edge_block"] for c in range(NCORES)], axis=0
    )
    node_mask = res.results[0]["node_mask"]
    return node_mask, edge_mask


# revision 14
# speedup vs baseline: 1.4499x; 1.0818x over previous
"""Trainium2 Bass kernel for nn_CausalMask (gumbel-sigmoid node/edge masks +
symmetric scatter into a [P, P] edge mask), SPMD across 8 NeuronCores.

Strategy (row-sharded scatter):
  - Core k owns rows [k*768, (k+1)*768) of the [6144, 6144] edge mask.
    Its block lives in SBUF as [128 partitions x 36864 values] (partition
    p holds mask rows 6p..6p+5 of the block, row-major).
  - The host routes each scattered entry (both (r,c) and (c,r) of every
    edge) to (core, partition, window, slot). A window is a column
    segment of one row.
  - On device: ACT/DVE compute the gumbel-sigmoid edge values for the
    padded entry buffer; gpsimd local_scatter calls compose the block
    (zeros + values fused, one window per call); DMAs stream the block
    to HBM, pipelined with the scatters.

Two composition modes:
  - exact (default): block in f32; each value lands as two adjacent int16
    scatters (bitcast trick), bit-exact f32. 48 windows of 768 cols.
  - fp16 (CM_FP16=1): block in fp16, upcast to f32 during the SWDGE
    output DMA. Half the gpsimd stream (24 windows of 1536 cols);
    max relative error 2^-11 (~4.9e-4) on scattered values.
"""

import os
import sys
import types

for _p in ("/opt/trn_rl_repo", "/root/.axon_site"):
    if _p not in sys.path:
        sys.path.insert(0, _p)

# NTFF profile hook (used only when BASS_TRACE=1): the image's antenv lacks
# axon_hooks, so provide it via sys.modules before bass_utils imports it.
if "antenv.axon_hooks" not in sys.modules:
    _m = types.ModuleType("antenv.axon_hooks")

    def _get_hook():
        try:
            from trn_agent_boot.trn_boot import _ntff_profile_via_ctypes

            return _ntff_profile_via_ctypes("/opt/axon/libaxon_pjrt.so")
        except Exception:
            return None

    _m.get_axon_ntff_profile_hook = _get_hook
    _m.set_axon_ntff_profile_hook = lambda h: None
    sys.modules["antenv.axon_hooks"] = _m

import numpy as np

P = 6144          # num_patches
E = 262144        # number of edges
NCORES = 8
RPB = P // NCORES     # 768 rows per core block
RPP = RPB // 128      # 6 mask rows per partition
BLKF = RPP * P        # 36864 values per partition
NCH = 8               # output DMA chunks (each = 4608 values/partition)
CHF = BLKF // NCH
TAU = 1.0
EPS = 1e-10

FP16 = os.environ.get("CM_FP16", "0") == "1"
if FP16:
    WF = 1536         # columns per scatter window
else:
    WF = 768
NCT = P // WF         # col tiles per row
W = RPP * NCT         # windows per partition
CHW = W // NCH        # windows per DMA chunk

_BUILD_CACHE: dict[tuple, object] = {}
LAST_RESULTS = None   # BassKernelResults of the most recent run (for test.py)


def _build_program(K: int):
    """Build + finalize the SPMD Bass program for per-cell slot count K."""
    import concourse.bacc as bacc
    import concourse.mybir as mybir
    import concourse.tile as tile

    f32 = mybir.dt.float32
    f16 = mybir.dt.float16
    i16 = mybir.dt.int16
    AF = mybir.ActivationFunctionType
    ALU = mybir.AluOpType
    LK = W * K
    NPF = P // 128  # 48 node values per partition
    IPW = K if FP16 else 2 * K    # idx int16s per window

    nc = bacc.Bacc()
    # register EPS as a const AP so activation(bias=EPS) resolves
    _ct = nc.alloc_sbuf_tensor(f"const-f32-eps", [128, 1], f32)
    nc.gpsimd.memset(_ct.ap(), EPS)
    nc.const_aps.aps[(f32, EPS)] = _ct.ap()
    nc.all_engine_barrier()

    CK = CHW * K          # padded entries per partition per chunk
    VSEG = 2 * CK         # int16 elems of el / eu segment per chunk
    ISEG = CHW * IPW      # int16 elems of idx segment per chunk
    CPK = 2 * VSEG + ISEG

    pk = nc.declare_dram_parameter("pk", [128, NCH * CPK], i16, isOutput=False)
    nl = nc.declare_dram_parameter("nl", [P], f32, isOutput=False)
    nu = nc.declare_dram_parameter("nu", [P], f32, isOutput=False)
    eb = nc.declare_dram_parameter("edge_block", [RPB, P], f32, isOutput=True)
    nm = nc.declare_dram_parameter("node_mask", [P], f32, isOutput=True)

    with tile.TileContext(nc) as tc:
        with tc.tile_pool(name="sbuf", bufs=1) as pool:
            tpk = pool.tile([128, NCH * CPK], i16, tag="tpk")
            if FP16:
                th = pool.tile([128, LK], f16, tag="th")  # downcast values

            def load_chunk(j):
                s = slice(j * CPK, (j + 1) * CPK)
                nc.scalar.dma_start(tpk[:, s], pk[:, s])

            load_chunk(0)
            load_chunk(1)

            # warm the ACT Ln/Sigmoid tables while chunk 0 is in flight
            warm = pool.tile([128, 1], f32, tag="warm")
            nc.scalar.activation(warm[:], _ct.ap(), AF.Ln, bias=EPS)
            nc.scalar.activation(warm[:], warm[:], AF.Sigmoid)

            # dummy scatter: forces the gpsimd library IRAM load to start
            # immediately instead of right before the first real window
            wi16 = pool.tile([128, 2], i16, tag="wi16")
            nc.gpsimd.memset(wi16[:], -1)
            nc.gpsimd.local_scatter(
                out_ap=warm[:].bitcast(i16),
                data_ap=wi16[:],
                idxs_ap=wi16[:],
                channels=128,
                num_elems=2,
                num_idxs=2,
            )

            # ---- per-chunk: compute gumbel-sigmoid values, compose windows
            #      via local_scatter, stream chunk to HBM
            ebf = eb[:, :].rearrange("(p a) b -> p (a b)", p=128)  # [128, BLKF]
            for j in range(NCH):
                if j + 2 < NCH:
                    load_chunk(j + 2)
                base = j * CPK
                elf = tpk[:, base : base + VSEG].bitcast(f32)
                euf = tpk[:, base + VSEG : base + 2 * VSEG].bitcast(f32)
                # g = -ln(-ln(u + eps) + eps);  v = sigmoid((logit + g) / tau)
                nc.scalar.activation(euf, euf, AF.Ln, bias=EPS)
                # guard: ln(u+eps) must stay <= 0 so -ln(..)+eps > 0
                nc.vector.tensor_scalar_min(euf, euf, 0.0)
                nc.scalar.activation(euf, euf, AF.Ln, bias=EPS, scale=-1.0)
                nc.vector.tensor_tensor(elf, elf, euf, op=ALU.subtract)
                nc.scalar.activation(elf, elf, AF.Sigmoid, scale=1.0 / TAU)
                if FP16:
                    hj = th[:, j * CK : (j + 1) * CK]
                    nc.vector.tensor_copy(hj, elf)

                blk = pool.tile([128, CHF], f16 if FP16 else f32, tag=f"blk{j}")
                for wi in range(CHW):
                    if FP16:
                        data = th[:, j * CK + wi * K : j * CK + (wi + 1) * K]
                        ne = WF
                    else:
                        data = tpk[:, base + wi * 2 * K : base + (wi + 1) * 2 * K]
                        ne = 2 * WF
                    nc.gpsimd.local_scatter(
                        out_ap=blk[:, wi * WF : (wi + 1) * WF].bitcast(i16),
                        data_ap=data,
                        idxs_ap=tpk[
                            :,
                            base + 2 * VSEG + wi * IPW : base + 2 * VSEG + (wi + 1) * IPW,
                        ],
                        channels=128,
                        num_elems=ne,
                        num_idxs=IPW,
                    )
                # last chunk: finest DMA granularity so the kernel-tail
                # drain waits on a small final transfer, not a whole chunk
                nsplit = CHW if j == NCH - 1 else (1 if FP16 else 2)
                step = CHF // nsplit
                for si in range(nsplit):
                    lo = j * CHF + si * step
                    dst = ebf[:, lo : lo + step]
                    src = blk[:, si * step : (si + 1) * step]
                    if FP16:
                        # SWDGE cast-DMA: fp16 SBUF -> f32 HBM
                        nc.gpsimd.dma_start(dst, src)
                    else:
                        nc.sync.dma_start(dst, src)

            # ---- node mask (identical on every core; tiny — run at the end
            #      so it never delays the scatter pipeline)
            tnl = pool.tile([128, NPF], f32, tag="tnl")
            tnu = pool.tile([128, NPF], f32, tag="tnu")
            nc.scalar.dma_start(tnl[:], nl[:].rearrange("(a b) -> a b", a=128))
            nc.scalar.dma_start(tnu[:], nu[:].rearrange("(a b) -> a b", a=128))
            nc.scalar.activation(tnu[:], tnu[:], AF.Ln, bias=EPS)
            nc.vector.tensor_scalar_min(tnu[:], tnu[:], 0.0)
            nc.scalar.activation(tnu[:], tnu[:], AF.Ln, bias=EPS, scale=-1.0)
            nc.vector.tensor_tensor(tnl[:], tnl[:], tnu[:], op=ALU.subtract)
            nc.scalar.activation(tnl[:], tnl[:], AF.Sigmoid, scale=1.0 / TAU)
            nc.sync.dma_start(nm[:].rearrange("(a b) -> a b", a=128), tnl[:])

    nc.finalize()
    return nc


def _route_entries(rows: np.ndarray, cols: np.ndarray):
    """Route 2E scattered entries to (core, partition, window, slot).

    Returns (K, dest, order, cpos): order indexes into the concatenated
    entry list (first E: (r,c), second E: (c,r)); dest is the flat slot
    index into the per-core padded buffers [NCORES, 128, W, K]; K is the
    global max entries per (core, partition, window) cell (even); cpos
    the in-window column position of every entry.
    """
    rr = np.concatenate([rows, cols]).astype(np.int64)
    cc = np.concatenate([cols, rows]).astype(np.int64)

    core = rr // RPB
    lr = rr - core * RPB
    p = lr // RPP
    q = lr - p * RPP
    ct = cc // WF
    cpos = cc - ct * WF
    w = q * NCT + ct
    cell = (core * 128 + p) * W + w

    order = np.argsort(cell, kind="stable")
    cell_s = cell[order]
    first = np.r_[0, np.flatnonzero(np.diff(cell_s)) + 1]
    counts = np.diff(np.r_[first, len(cell_s)])
    K = int(counts.max())
    K += K & 1  # num_idxs must be even in fp16 mode
    slot = np.arange(len(cell_s), dtype=np.int64) - np.repeat(first, counts)
    dest = cell_s * K + slot
    return K, dest, order, cpos


def kernel(node_logits, edge_logits, u_node, u_edge, rows, cols):
    global LAST_RESULTS
    from concourse.bass_utils import run_bass_kernel_spmd

    node_logits = np.asarray(node_logits, np.float32)
    edge_logits = np.asarray(edge_logits, np.float32)
    u_node = np.asarray(u_node, np.float32)
    u_edge = np.asarray(u_edge, np.float32)
    rows = np.asarray(rows)
    cols = np.asarray(cols)

    K, dest, order, cpos = _route_entries(rows, cols)

    nc = _BUILD_CACHE.get((K, FP16))
    if nc is None:
        nc = _build_program(K)
        _BUILD_CACHE[(K, FP16)] = nc

    # padded per-core buffers (padding never scattered: idx = -1; u=0
    # padding is safe through the clamped log-log pipeline)
    ncell = NCORES * 128 * W
    el_pad = np.zeros(ncell * K, np.float32)
    eu_pad = np.zeros(ncell * K, np.float32)
    IPW = K if FP16 else 2 * K
    ei_pad = np.full(ncell * IPW, -1, np.int16)

    ee = np.concatenate([np.arange(E), np.arange(E)])[order]
    el_pad[dest] = edge_logits[ee]
    eu_pad[dest] = u_edge[ee]
    cpos_s = cpos[order]
    if FP16:
        ei_pad[dest] = cpos_s.astype(np.int16)
    else:
        ei_pad[2 * dest] = (2 * cpos_s).astype(np.int16)
        ei_pad[2 * dest + 1] = (2 * cpos_s + 1).astype(np.int16)

    # pack [el | eu | ei] per chunk (int16 view, chunk-contiguous)
    VSEG = 2 * CHW * K
    ISEG = CHW * IPW
    el16 = el_pad.view(np.int16).reshape(NCORES, 128, NCH, VSEG)
    eu16 = eu_pad.view(np.int16).reshape(NCORES, 128, NCH, VSEG)
    ei16 = ei_pad.reshape(NCORES, 128, NCH, ISEG)
    CPK = 2 * VSEG + ISEG
    pk = np.empty((NCORES, 128, NCH, CPK), np.int16)
    pk[..., :VSEG] = el16
    pk[..., VSEG : 2 * VSEG] = eu16
    pk[..., 2 * VSEG :] = ei16
    pk = pk.reshape(NCORES, 128, NCH * CPK)

    in_maps = [
        {"pk": pk[c], "nl": node_logits, "nu": u_node} for c in range(NCORES)
    ]

    res = run_bass_kernel_spmd(nc, in_maps, list(range(NCORES)))
    LAST_RESULTS = res

    edge_mask = np.concatenate(
        [res.results[c]["edge_block"] for c in range(NCORES)], axis=0
    )
    node_mask = res.results[0]["node_mask"]
    return node_mask, edge_mask


# revision 15
# speedup vs baseline: 1.4686x; 1.0129x over previous
"""Trainium2 Bass kernel for nn_CausalMask (gumbel-sigmoid node/edge masks +
symmetric scatter into a [P, P] edge mask), SPMD across 8 NeuronCores.

Strategy (row-sharded scatter):
  - Core k owns rows [k*768, (k+1)*768) of the [6144, 6144] edge mask.
    Its block lives in SBUF as [128 partitions x 36864 values] (partition
    p holds mask rows 6p..6p+5 of the block, row-major).
  - The host routes each scattered entry (both (r,c) and (c,r) of every
    edge) to (core, partition, window, slot). A window is a contiguous
    2046-byte-pair segment of the partition's flat 36864-value image
    (windows may span mask-row boundaries; 2046 is the gpsimd
    local_scatter scratch limit).
  - On device: ACT/DVE compute the gumbel-sigmoid edge values for the
    padded entry buffer; gpsimd local_scatter calls compose the block
    image (zeros + values fused, one window per call); DMAs stream the
    image to HBM, pipelined with the scatters.

Two composition modes:
  - exact (default): block in f32; each value lands as two adjacent int16
    scatters (bitcast trick), bit-exact f32. 37 windows.
  - fp16 (CM_FP16=1): block in fp16, upcast to f32 during the SWDGE
    output DMA. Half the gpsimd stream (19 windows); max relative error
    2^-11 (~4.9e-4) on scattered values, ~30% faster end to end.
"""

import os
import sys
import types

for _p in ("/opt/trn_rl_repo", "/root/.axon_site"):
    if _p not in sys.path:
        sys.path.insert(0, _p)

# NTFF profile hook (used only when BASS_TRACE=1): the image's antenv lacks
# axon_hooks, so provide it via sys.modules before bass_utils imports it.
if "antenv.axon_hooks" not in sys.modules:
    _m = types.ModuleType("antenv.axon_hooks")

    def _get_hook():
        try:
            from trn_agent_boot.trn_boot import _ntff_profile_via_ctypes

            return _ntff_profile_via_ctypes("/opt/axon/libaxon_pjrt.so")
        except Exception:
            return None

    _m.get_axon_ntff_profile_hook = _get_hook
    _m.set_axon_ntff_profile_hook = lambda h: None
    sys.modules["antenv.axon_hooks"] = _m

import numpy as np

P = 6144          # num_patches
E = 262144        # number of edges
NCORES = 8
RPB = P // NCORES     # 768 rows per core block
RPP = RPB // 128      # 6 mask rows per partition
BLKF = RPP * P        # 36864 values per partition
NCH = 8               # output DMA chunks
CHF = BLKF // NCH     # 4608 values per chunk per partition
TAU = 1.0
EPS = 1e-10
MAXNE = 2046          # local_scatter num_elems limit (2-byte elems)

FP16 = os.environ.get("CM_FP16", "0") == "1"
EUNIT = 1 if FP16 else 2                  # 2-byte elems per value
BLK2 = BLKF * EUNIT                       # 2-byte elems per partition image
WN = -(-BLK2 // MAXNE)                    # number of scatter windows
WLEN = [min(MAXNE, BLK2 - w * MAXNE) for w in range(WN)]

# input groups (windows per group): small first group for a fast
# pipeline start, the rest spread evenly
NG = 8


def _group_bounds():
    first = min(2, WN)
    rest = WN - first
    ngr = NG - 1
    sizes = [first] + [rest // ngr + (1 if g < rest % ngr else 0) for g in range(ngr)]
    gb = [0]
    for s in sizes:
        gb.append(gb[-1] + s)
    return gb


GB = _group_bounds()

_BUILD_CACHE: dict[tuple, object] = {}
LAST_RESULTS = None   # BassKernelResults of the most recent run (for test.py)


def _build_program(K: int):
    """Build + finalize the SPMD Bass program for per-cell slot count K."""
    import concourse.bacc as bacc
    import concourse.mybir as mybir
    import concourse.tile as tile

    f32 = mybir.dt.float32
    f16 = mybir.dt.float16
    i16 = mybir.dt.int16
    AF = mybir.ActivationFunctionType
    ALU = mybir.AluOpType
    NPF = P // 128            # 48 node values per partition
    IPW = EUNIT * K           # idx int16s per window
    # per-group packed segment sizes (int16 units)
    NW = [GB[g + 1] - GB[g] for g in range(NG)]
    VSEG = [2 * K * n for n in NW]         # el / eu (f32 as int16 pairs)
    ISEG = [IPW * n for n in NW]
    CPK = [2 * v + i for v, i in zip(VSEG, ISEG)]
    GOFF = np.concatenate([[0], np.cumsum(CPK)]).astype(int)
    PKN = int(GOFF[-1])

    nc = bacc.Bacc()
    # register EPS as a const AP so activation(bias=EPS) resolves
    _ct = nc.alloc_sbuf_tensor(f"const-f32-eps", [128, 1], f32)
    nc.gpsimd.memset(_ct.ap(), EPS)
    nc.const_aps.aps[(f32, EPS)] = _ct.ap()
    nc.all_engine_barrier()

    pk = nc.declare_dram_parameter("pk", [128, PKN], i16, isOutput=False)
    nl = nc.declare_dram_parameter("nl", [P], f32, isOutput=False)
    nu = nc.declare_dram_parameter("nu", [P], f32, isOutput=False)
    eb = nc.declare_dram_parameter("edge_block", [RPB, P], f32, isOutput=True)
    nm = nc.declare_dram_parameter("node_mask", [P], f32, isOutput=True)

    with tile.TileContext(nc) as tc:
        with tc.tile_pool(name="sbuf", bufs=1) as pool:
            tpk = pool.tile([128, PKN], i16, tag="tpk")
            blk = pool.tile([128, BLKF], f16 if FP16 else f32, tag="blk")
            if FP16:
                th = pool.tile([128, WN * K], f16, tag="th")  # downcast values

            def load_group(g):
                s = slice(int(GOFF[g]), int(GOFF[g + 1]))
                nc.scalar.dma_start(tpk[:, s], pk[:, s])

            load_group(0)
            load_group(1)

            # warm the ACT Ln/Sigmoid tables while group 0 is in flight
            warm = pool.tile([128, 1], f32, tag="warm")
            nc.scalar.activation(warm[:], _ct.ap(), AF.Ln, bias=EPS)
            nc.scalar.activation(warm[:], warm[:], AF.Sigmoid)

            # dummy scatter: forces the gpsimd library IRAM load to start
            # immediately instead of right before the first real window
            wi16 = pool.tile([128, 2], i16, tag="wi16")
            nc.gpsimd.memset(wi16[:], -1)
            nc.gpsimd.local_scatter(
                out_ap=warm[:].bitcast(i16),
                data_ap=wi16[:],
                idxs_ap=wi16[:],
                channels=128,
                num_elems=2,
                num_idxs=2,
            )

            ebf = eb[:, :].rearrange("(p a) b -> p (a b)", p=128)  # [128, BLKF]

            def out_chunk(c):
                # last chunk at finest granularity: the kernel-tail drain
                # then waits on a small final transfer
                nsplit = 3 if c == NCH - 1 else 1
                step = CHF // nsplit
                for si in range(nsplit):
                    lo = c * CHF + si * step
                    dst = ebf[:, lo : lo + step]
                    src = blk[:, lo : lo + step]
                    if FP16:
                        nc.gpsimd.dma_start(dst, src)  # SWDGE cast fp16->f32
                    else:
                        nc.sync.dma_start(dst, src)

            next_chunk = 0
            for g in range(NG):
                if g + 2 < NG:
                    load_group(g + 2)
                base = int(GOFF[g])
                vs, isg, nw = VSEG[g], ISEG[g], NW[g]
                elf = tpk[:, base : base + vs].bitcast(f32)
                euf = tpk[:, base + vs : base + 2 * vs].bitcast(f32)
                # g = -ln(-ln(u + eps) + eps);  v = sigmoid((logit + g)/tau)
                nc.scalar.activation(euf, euf, AF.Ln, bias=EPS)
                # guard: ln(u+eps) must stay <= 0 so -ln(..)+eps > 0
                nc.vector.tensor_scalar_min(euf, euf, 0.0)
                nc.scalar.activation(euf, euf, AF.Ln, bias=EPS, scale=-1.0)
                nc.vector.tensor_tensor(elf, elf, euf, op=ALU.subtract)
                nc.scalar.activation(elf, elf, AF.Sigmoid, scale=1.0 / TAU)
                if FP16:
                    hg = th[:, GB[g] * K : GB[g + 1] * K]
                    nc.vector.tensor_copy(hg, elf)

                for wl in range(nw):
                    w = GB[g] + wl
                    if FP16:
                        data = th[:, w * K : (w + 1) * K]
                    else:
                        data = tpk[:, base + wl * 2 * K : base + (wl + 1) * 2 * K]
                    start2 = w * MAXNE  # window start in 2-byte elems
                    nc.gpsimd.local_scatter(
                        out_ap=blk[:].bitcast(i16)[:, start2 : start2 + WLEN[w]],
                        data_ap=data,
                        idxs_ap=tpk[
                            :, base + 2 * vs + wl * IPW : base + 2 * vs + (wl + 1) * IPW
                        ],
                        channels=128,
                        num_elems=WLEN[w],
                        num_idxs=IPW,
                    )
                    # issue chunk DMAs as soon as their windows are done
                    while (
                        next_chunk < NCH
                        and (w + 1) * MAXNE >= (next_chunk + 1) * CHF * EUNIT
                    ):
                        out_chunk(next_chunk)
                        next_chunk += 1
            while next_chunk < NCH:
                out_chunk(next_chunk)
                next_chunk += 1

            # ---- node mask (identical on every core; tiny — at the end so
            #      it never delays the scatter pipeline)
            tnl = pool.tile([128, NPF], f32, tag="tnl")
            tnu = pool.tile([128, NPF], f32, tag="tnu")
            nc.scalar.dma_start(tnl[:], nl[:].rearrange("(a b) -> a b", a=128))
            nc.scalar.dma_start(tnu[:], nu[:].rearrange("(a b) -> a b", a=128))
            nc.scalar.activation(tnu[:], tnu[:], AF.Ln, bias=EPS)
            nc.vector.tensor_scalar_min(tnu[:], tnu[:], 0.0)
            nc.scalar.activation(tnu[:], tnu[:], AF.Ln, bias=EPS, scale=-1.0)
            nc.vector.tensor_tensor(tnl[:], tnl[:], tnu[:], op=ALU.subtract)
            nc.scalar.activation(tnl[:], tnl[:], AF.Sigmoid, scale=1.0 / TAU)
            nc.sync.dma_start(nm[:].rearrange("(a b) -> a b", a=128), tnl[:])

    nc.finalize()
    return nc


def _route_entries(rows: np.ndarray, cols: np.ndarray):
    """Route 2E scattered entries to (core, partition, window, slot).

    Returns (K, dest, order, cpos2): order indexes into the concatenated
    entry list (first E: (r,c), second E: (c,r)); dest is the flat slot
    index into the per-core padded buffers [NCORES, 128, WN, K]; K is the
    global max entries per (core, partition, window) cell (even); cpos2
    the in-window 2-byte-elem position of every entry.
    """
    rr = np.concatenate([rows, cols]).astype(np.int64)
    cc = np.concatenate([cols, rows]).astype(np.int64)

    core = rr // RPB
    lr = rr - core * RPB
    p = lr // RPP
    q = lr - p * RPP
    flat = q * P + cc                  # value position in partition image
    e2 = flat * EUNIT                  # 2-byte-elem position
    w = e2 // MAXNE
    cpos2 = e2 - w * MAXNE
    cell = (core * 128 + p) * WN + w

    order = np.argsort(cell, kind="stable")
    cell_s = cell[order]
    first = np.r_[0, np.flatnonzero(np.diff(cell_s)) + 1]
    counts = np.diff(np.r_[first, len(cell_s)])
    K = int(counts.max())
    K += K & 1  # num_idxs must be even in fp16 mode
    slot = np.arange(len(cell_s), dtype=np.int64) - np.repeat(first, counts)
    dest = cell_s * K + slot
    return K, dest, order, cpos2


def kernel(node_logits, edge_logits, u_node, u_edge, rows, cols):
    global LAST_RESULTS
    from concourse.bass_utils import run_bass_kernel_spmd

    node_logits = np.asarray(node_logits, np.float32)
    edge_logits = np.asarray(edge_logits, np.float32)
    u_node = np.asarray(u_node, np.float32)
    u_edge = np.asarray(u_edge, np.float32)
    rows = np.asarray(rows)
    cols = np.asarray(cols)

    K, dest, order, cpos2 = _route_entries(rows, cols)

    nc = _BUILD_CACHE.get((K, FP16))
    if nc is None:
        nc = _build_program(K)
        _BUILD_CACHE[(K, FP16)] = nc

    # padded per-core buffers (padding never scattered: idx = -1; u=0
    # padding is safe through the clamped log-log pipeline)
    ncell = NCORES * 128 * WN
    IPW = EUNIT * K
    el_pad = np.zeros(ncell * K, np.float32)
    eu_pad = np.zeros(ncell * K, np.float32)
    ei_pad = np.full(ncell * IPW, -1, np.int16)

    ee = np.concatenate([np.arange(E), np.arange(E)])[order]
    el_pad[dest] = edge_logits[ee]
    eu_pad[dest] = u_edge[ee]
    cpos_s = cpos2[order]
    if FP16:
        ei_pad[dest] = cpos_s.astype(np.int16)
    else:
        ei_pad[2 * dest] = cpos_s.astype(np.int16)
        ei_pad[2 * dest + 1] = (cpos_s + 1).astype(np.int16)

    # pack [el | eu | ei] per window group (int16 view, group-contiguous)
    el16 = el_pad.view(np.int16).reshape(NCORES, 128, WN, 2 * K)
    eu16 = eu_pad.view(np.int16).reshape(NCORES, 128, WN, 2 * K)
    ei16 = ei_pad.reshape(NCORES, 128, WN, IPW)
    segs = []
    for g in range(NG):
        ws = slice(GB[g], GB[g + 1])
        nwg = GB[g + 1] - GB[g]
        segs.append(el16[:, :, ws].reshape(NCORES, 128, nwg * 2 * K))
        segs.append(eu16[:, :, ws].reshape(NCORES, 128, nwg * 2 * K))
        segs.append(ei16[:, :, ws].reshape(NCORES, 128, nwg * IPW))
    pk = np.concatenate(segs, axis=2)

    in_maps = [
        {"pk": pk[c], "nl": node_logits, "nu": u_node} for c in range(NCORES)
    ]

    res = run_bass_kernel_spmd(nc, in_maps, list(range(NCORES)))
    LAST_RESULTS = res

    edge_mask = np.concatenate(
        [res.results[c]["edge_block"] for c in range(NCORES)], axis=0
    )
    node_mask = res.results[0]["node_mask"]
    return node_mask, edge_mask
